# revision 1
# baseline (speedup 1.0000x reference)
"""Trainium2 Bass kernel for nn_CachedShapingFunctions (embedding_lookup).

out[b,t,w] = interp of lookup_table[:, w] at uniform-bucket position of
inputs[b,t,w].  Data-parallel over batch across 8 NeuronCores; the LUT is
replicated (as a host-prepared (value, delta) pair table per waveshaper).

Per-core pipeline (shard = [8192? no: 2 batches x 32768 t x 64 w] flattened
to [65536, 64]):
  - DMA in natural tiles, XBAR DMA-transpose (as 2x u16 planes) to
    waveshaper-on-partition layout [128 = 2 time-chunks x 64 w, 512 t]
  - DVE arithmetic: exact floor/clamp of the continuous bucket position
  - GPSIMD ap_gather of (value, delta) pairs from a per-partition table
  - strided extraction of the 1/16-dense gather output, interpolation
  - XBAR transpose back, DMA out
"""
import sys
import numpy as np

sys.path.insert(0, '/opt/trn_rl_repo')

import bass_rust
import concourse.bass as bass
import concourse.mybir as mybir
import concourse.tile as tile_mod
from concourse.tile import TileContext
from concourse.vector_clock import ScopedClock
from concourse import library_config

MIN_VALUE, MAX_VALUE = -3.0, 3.0
NB = 4096          # buckets
W = 64             # waveshapers
N_CORES = 8

# ---------------------------------------------------------------- patches --
# This walrus build accepts at most ONE sync-wait per instruction.  The Tile
# tail drain and scheduler can attach more; spill the excess onto nops.

_MAXW = 1

def _spill_waits(nc):
    for f in nc.m.functions:
        for bb in f.blocks:
            out = []
            for inst in list(bb.instructions):
                si = inst.sync_info
                if si is not None and len(si.on_wait) > _MAXW:
                    waits = list(si.on_wait)
                    spill = waits[:-_MAXW]
                    for i in range(0, len(spill), _MAXW):
                        nop = mybir.InstNoOp(
                            name=f"wspill_{inst.name}_{i}", ins=[], outs=[])
                        nop.engine = inst.engine
                        nop.sync_info = bass_rust.SyncInfo(
                            on_wait=spill[i:i + _MAXW], on_update=[])
                        out.append(nop)
                    inst.sync_info = bass_rust.SyncInfo(
                        on_wait=waits[-_MAXW:], on_update=list(si.on_update))
                out.append(inst)
            bb.instructions = out


def _patched_drain_and_barrier(self, tick_clock, wait_clock):
    nc = self.nc
    drain_inst = nc.sync.drain()
    wait_clock.add_sem_waits(
        drain_inst.ins, ScopedClock({None: tick_clock.global_clock}))
    si = drain_inst.ins.sync_info
    if si is not None and len(si.on_wait) > _MAXW:
        waits = list(si.on_wait)
        drain_inst.ins.sync_info = bass_rust.SyncInfo(
            on_wait=waits[:_MAXW], on_update=list(si.on_update))
        rest = waits[_MAXW:]
        for i in range(0, len(rest), _MAXW):
            nop = nc.sync.nop(hint="drain_wait_spill", nofuse=True)
            nop.ins.sync_info = bass_rust.SyncInfo(
                on_wait=rest[i:i + _MAXW], on_update=[])
    nc.all_engine_barrier()
    assert self.sems is not None
    popped = nc._tile_sem_poison_stack.pop()
    assert popped is self._sem_poison
    nc.clear_and_free_semaphores(list(self.sems.allocated().values()))
    nc.all_engine_barrier()


tile_mod.TileContext._drain_and_barrier = _patched_drain_and_barrier

# ----------------------------------------------------------------- kernel --

S = 512            # t-columns per transposed super-tile (per chunk)
TROWS = 2 * S      # natural t rows covered per super-tile (2 chunks)

F32 = mybir.dt.float32
I32 = mybir.dt.int32
I16 = mybir.dt.int16
U16 = mybir.dt.uint16


def build_kernel(n_rows):
    """n_rows: flattened time rows per core (65536 full scale)."""
    assert n_rows % TROWS == 0
    n_tiles = n_rows // TROWS
    nc = bass.Bass()
    x_d = nc.dram_tensor("x", [n_rows, W], F32, kind="ExternalInput")
    tbl_d = nc.dram_tensor("tbl", [128, NB * 2 + 16 + 128], F32, kind="ExternalInput")
    y_d = nc.dram_tensor("y", [n_rows, W], F32, kind="ExternalOutput")

    with TileContext(nc) as tc:
        with (
            tc.tile_pool(name="const", bufs=1) as cpool,
            tc.tile_pool(name="io", bufs=2) as iop,
            tc.tile_pool(name="tp", bufs=2) as tpp,
            tc.tile_pool(name="sc", bufs=1) as scp,
            tc.tile_pool(name="sc2", bufs=2) as scp2,
            tc.tile_pool(name="sp", bufs=2) as spp,
            tc.tile_pool(name="ps", bufs=2, space="PSUM") as psp,
        ):
            tbl = cpool.tile([128, NB * 2 + 16 + 128], F32)
            nc.sync.dma_start(tbl[:, :], tbl_d[:, :])
            nc.gpsimd.load_library(library_config.ap_gather)
            tbl3 = tbl[:, :NB * 2].rearrange("p (n d) -> p n d", d=2)
            mask = tbl[:, NB * 2: NB * 2 + 16]
            ident = tbl[:, NB * 2 + 16:]

            def emit_head(it):
                base = it * TROWS * W
                # partition p loads 8 consecutive rows = one contiguous
                # 2KB line (vs 8x 256B segments); the symmetric output AP
                # cancels the resulting t-permutation.
                xnat = iop.tile([128, 8 * W], F32, tag="xnat")
                in_ap = bass.AP(x_d, base, [[8 * W, 128], [1, 8 * W]])
                nc.sync.dma_start(xnat[:, :], in_ap)

                xT = tpp.tile([128, S], F32, tag="xT")
                for k in range(4):
                    pst = psp.tile([128, 128], F32, tag="psin")
                    nc.tensor.transpose(
                        pst[:, :], xnat[:, 128 * k: 128 * k + 128], ident)
                    nc.scalar.copy(xT[:, 128 * k: 128 * k + 128], pst[:, :])

                ic = scp.tile([128, S], F32, tag="ic")
                icc = scp.tile([128, S], F32, tag="icc")
                ili = scp.tile([128, S], I16, tag="ili")
                ilf = scp.tile([128, S], F32, tag="ilf")
                fd = scp.tile([128, S], F32, tag="fx")
                il2 = scp.tile([128, S], F32, tag="il2")
                dd = scp.tile([128, S], F32, tag="fx")
                fu = scp.tile([128, S], F32, tag="fu")
                il3 = scp.tile([128, S], F32, tag="ilf")
                ff = scp2.tile([128, S], F32, tag="ff")
                idx = scp2.tile([128, S], I16, tag="idx")

                A = mybir.AluOpType
                nc.vector.tensor_scalar(ic[:, :], xT[:, :], 3.0, 682.5, A.add, A.mult)
                nc.vector.tensor_scalar(icc[:, :], ic[:, :], 0.0, 4095.0, A.max, A.min)
                nc.vector.tensor_copy(ili[:, :], icc[:, :])
                nc.vector.tensor_copy(ilf[:, :], ili[:, :])
                nc.vector.tensor_tensor(fd[:, :], ilf[:, :], icc[:, :], A.is_gt)
                nc.vector.tensor_tensor(il2[:, :], ilf[:, :], fd[:, :], A.subtract)
                nc.vector.tensor_tensor(dd[:, :], icc[:, :], il2[:, :], A.subtract)
                nc.vector.tensor_scalar(fu[:, :], dd[:, :], 1.0, None, A.is_ge)
                nc.vector.tensor_tensor(il3[:, :], il2[:, :], fu[:, :], A.add)
                nc.vector.tensor_tensor(ff[:, :], ic[:, :], il3[:, :], A.subtract)
                nc.vector.tensor_copy(idx[:, :], il3[:, :])

                sparse = spp.tile([128, 16 * S * 2], F32, tag="sparse")
                sp3 = sparse[:, :].rearrange("p (n d) -> p n d", d=2)
                nc.gpsimd.ap_gather(sp3, tbl3, idx[:, :], channels=128,
                                    num_elems=NB, d=2, num_idxs=16 * S)
                return sparse, ff

            def emit_tail(it, sparse, ff):
                base = it * TROWS * W
                A = mybir.AluOpType
                sp3 = sparse[:, :].rearrange("p (n d) -> p n d", d=2)
                pairs = scp.tile([128, S * 2], F32, tag="pairs")
                pr3 = pairs[:, :].rearrange("p (n d) -> p n d", d=2)
                for r in range(16):
                    srcv = sparse[:, :].rearrange(
                        "p (n q) -> p n q", q=32)[:, :, 2 * r: 2 * r + 2]
                    mcol = mask[:, r: r + 1]
                    if r == 0:
                        nc.vector.tensor_scalar(
                            pr3, srcv, mcol, None, A.mult)
                    else:
                        nc.vector.scalar_tensor_tensor(
                            pr3, srcv, mcol, pr3, A.mult, A.add)

                outT = tpp.tile([128, S], F32, tag="outT")
                nc.vector.tensor_tensor(outT[:, :], pr3[:, :, 1], ff[:, :], A.mult)
                nc.vector.tensor_tensor(outT[:, :], outT[:, :], pr3[:, :, 0], A.add)

                onat = iop.tile([128, 8 * W], F32, tag="onat")
                for k in range(4):
                    pst = psp.tile([128, 128], F32, tag="psout")
                    nc.tensor.transpose(
                        pst[:, :], outT[:, 128 * k: 128 * k + 128], ident)
                    nc.scalar.copy(onat[:, 128 * k: 128 * k + 128], pst[:, :])

                out_ap = bass.AP(y_d, base, [[8 * W, 128], [1, 8 * W]])
                nc.sync.dma_start(out_ap, onat[:, :])

            pending = None
            for it in range(n_tiles):
                sparse, ff = emit_head(it)
                if pending is not None:
                    emit_tail(*pending)
                pending = (it, sparse, ff)
            emit_tail(*pending)

    from concourse.library_overlay import lower_extended_insts
    lower_extended_insts(nc)
    _spill_waits(nc)
    return nc


def make_table(lookup_table):
    lut = np.asarray(lookup_table, dtype=np.float32)          # [4096, 64]
    vu = np.concatenate([lut[1:], lut[-1:]], axis=0)          # T[min(i+1,4095)]
    delta = vu - lut                                          # f32 exact
    pair = np.stack([lut, delta], axis=-1)                    # [4096, 64, 2]
    tblw = np.ascontiguousarray(pair.transpose(1, 0, 2)).reshape(W, NB * 2)
    tbl128 = np.concatenate([tblw, tblw], axis=0)             # [128, 8192]
    p = np.arange(128)
    m = (p[:, None] % 16 == np.arange(16)[None, :]).astype(np.float32)
    eye = np.eye(128, dtype=np.float32)
    return np.concatenate([tbl128, m, eye], axis=1)           # [128, 8336]


_CACHE = {}


def kernel(inputs, lookup_table):
    x = np.ascontiguousarray(np.asarray(inputs, dtype=np.float32))
    B, T, Wx = x.shape
    assert Wx == W
    per_core_b = B // N_CORES
    n_rows = per_core_b * T
    tbl = make_table(lookup_table)

    if n_rows not in _CACHE:
        _CACHE[n_rows] = build_kernel(n_rows)
    nc = _CACHE[n_rows]

    from concourse import bass_utils
    shards = x.reshape(N_CORES, n_rows, W)
    in_maps = [{"x": shards[c], "tbl": tbl} for c in range(N_CORES)]
    res = bass_utils.run_bass_kernel_spmd(
        nc, in_maps, core_ids=list(range(N_CORES)))
    out = np.stack([res.results[c]["y"] for c in range(N_CORES)], axis=0)
    return out.reshape(B, T, W)



# revision 3
# speedup vs baseline: 1.1701x; 1.1701x over previous
"""Trainium2 Bass kernel for nn_CachedShapingFunctions (embedding_lookup), v2.

out[b,t,w] = lerp of lookup_table[:, w] at uniform-bucket position of
inputs[b,t,w].  Data-parallel over batch across 8 NeuronCores.

v2 replaces the GPSIMD ap_gather (command-latency bound, ~97 cyc/elem) with a
TensorEngine one-hot gather.  Decompose il = 64*h + 64?no: il = 64*h + l with
h,l in [0,64).  Per waveshaper w (and a pair of 512-col time chunks packed
into the 128-partition dim):

  BH   = SEL_w.T @ htile          # PE: broadcast h over 64 partitions x2
  Sq   = Square(BH - iota)        # ACT
  OH   = (Sq < 0.5)               # DVE (bf16 4x) -> exact one-hot of h
  Yv   = S1v_w.T @ OH             # PE: 64 candidate values T[64h + l'] all l'
  Yd   = S1d_w.T @ OH             # PE: same for delta table
  BL   = SEL_w.T @ ltile          # PE: broadcast l
  Zv   = (BL == iota) * Yv        # DVE fused compare-mult (l-select mask)
  Zd   = (BL == iota) * Yd
  Vacc += RED_w.T @ Zv            # PE: partition-reduce + place into row w
  Dacc += RED_w.T @ Zd            #     (PSUM accumulation across the 64 w's)

final: out = Vacc + f * Dacc  (f = fractional position, fp32)
"""
import sys
import numpy as np

sys.path.insert(0, '/opt/trn_rl_repo')

import bass_rust
import concourse.bass as bass
import concourse.mybir as mybir
import concourse.tile as tile_mod
from concourse.tile import TileContext
from concourse.vector_clock import ScopedClock

MIN_VALUE, MAX_VALUE = -3.0, 3.0
NB = 4096          # buckets
W = 64             # waveshapers
N_CORES = 8

# ---------------------------------------------------------------- patches --
# This walrus build accepts at most ONE sync-wait per instruction.  The Tile
# tail drain and scheduler can attach more; spill the excess onto nops.

_MAXW = 1

def _spill_waits(nc):
    for f in nc.m.functions:
        for bb in f.blocks:
            out = []
            for inst in list(bb.instructions):
                si = inst.sync_info
                if si is not None and len(si.on_wait) > _MAXW:
                    waits = list(si.on_wait)
                    spill = waits[:-_MAXW]
                    for i in range(0, len(spill), _MAXW):
                        nop = mybir.InstNoOp(
                            name=f"wspill_{inst.name}_{i}", ins=[], outs=[])
                        nop.engine = inst.engine
                        nop.sync_info = bass_rust.SyncInfo(
                            on_wait=spill[i:i + _MAXW], on_update=[])
                        out.append(nop)
                    inst.sync_info = bass_rust.SyncInfo(
                        on_wait=waits[-_MAXW:], on_update=list(si.on_update))
                out.append(inst)
            bb.instructions = out


def _patched_drain_and_barrier(self, tick_clock, wait_clock):
    nc = self.nc
    drain_inst = nc.sync.drain()
    wait_clock.add_sem_waits(
        drain_inst.ins, ScopedClock({None: tick_clock.global_clock}))
    si = drain_inst.ins.sync_info
    if si is not None and len(si.on_wait) > _MAXW:
        waits = list(si.on_wait)
        drain_inst.ins.sync_info = bass_rust.SyncInfo(
            on_wait=waits[:_MAXW], on_update=list(si.on_update))
        rest = waits[_MAXW:]
        for i in range(0, len(rest), _MAXW):
            nop = nc.sync.nop(hint="drain_wait_spill", nofuse=True)
            nop.ins.sync_info = bass_rust.SyncInfo(
                on_wait=rest[i:i + _MAXW], on_update=[])
    nc.all_engine_barrier()
    assert self.sems is not None
    popped = nc._tile_sem_poison_stack.pop()
    assert popped is self._sem_poison
    nc.clear_and_free_semaphores(list(self.sems.allocated().values()))
    nc.all_engine_barrier()


tile_mod.TileContext._drain_and_barrier = _patched_drain_and_barrier

# ----------------------------------------------------------------- kernel --

S = 512            # t-columns per transposed super-tile (per chunk)
TROWS = 2 * S      # natural t rows covered per super-tile (2 chunks)

F32 = mybir.dt.float32
BF16 = mybir.dt.bfloat16
I16 = mybir.dt.int16

# bf16 stationaries table layout (free-dim offsets, per 128-col block):
#   SEL  at block w           (w in 0..63)
#   S1v  at block 64 + w
#   S1d  at block 128 + w
#   RED  at block 192 + w
TBL_BLOCKS = 256
GROUP = 8          # tilepairs python-unrolled inside the hardware For_i loop


def build_kernel(n_rows, spill=True, group=None):
    """n_rows: flattened time rows per core (65536 full scale).

    group=None: fully unroll all tiles (python loop).
    group=k: emit k python-unrolled tiles inside a hardware For_i loop.
    """
    assert n_rows % TROWS == 0
    n_tiles = n_rows // TROWS
    nc = bass.Bass()
    x_d = nc.dram_tensor("x", [n_rows, W], F32, kind="ExternalInput")
    tf_d = nc.dram_tensor("tf", [128, 128 + 2], F32, kind="ExternalInput")
    tb_d = nc.dram_tensor("tb", [128, TBL_BLOCKS * 128], BF16,
                          kind="ExternalInput")
    y_d = nc.dram_tensor("y", [n_rows, W], F32, kind="ExternalOutput")

    A = mybir.AluOpType
    AF = mybir.ActivationFunctionType

    with TileContext(nc) as tc:
        with (
            tc.tile_pool(name="const", bufs=1) as cpool,
            tc.tile_pool(name="io", bufs=2) as iop,
            tc.tile_pool(name="tp", bufs=2) as tpp,
            tc.tile_pool(name="sc", bufs=1) as scp,
            tc.tile_pool(name="sc2", bufs=2) as scp2,
            tc.tile_pool(name="sb", bufs=2) as sbp,
            tc.tile_pool(name="sb3", bufs=3) as sbp3,
            tc.tile_pool(name="psA", bufs=1, space="PSUM") as psA,
            tc.tile_pool(name="psL", bufs=1, space="PSUM") as psL,
            tc.tile_pool(name="psY", bufs=2, space="PSUM") as psY,
            tc.tile_pool(name="psY2", bufs=2, space="PSUM") as psY2,
            tc.tile_pool(name="acc", bufs=1, space="PSUM") as accp,
        ):
            tf = cpool.tile([128, 128 + 2], F32)
            nc.sync.dma_start(tf[:, :], tf_d[:, :])
            tb = cpool.tile([128, TBL_BLOCKS * 128], BF16)
            nc.sync.dma_start(tb[:, :], tb_d[:, :])
            ident = tf[:, 0:128]
            iota = tf[:, 128:129]       # p % 64, fp32
            negiota = tf[:, 129:130]    # -(p % 64), fp32

            def blk(b, w):
                off = (b * 64 + w) * 128
                return tb[:, off:off + 128]

            def emit_tile(base):
                # partition p loads 8 consecutive rows = one contiguous
                # 2KB line; the symmetric output AP cancels the resulting
                # t-permutation.
                xnat = iop.tile([128, 8 * W], F32, tag="xnat")
                in_ap = bass.AP(x_d, base, [[8 * W, 128], [1, 8 * W]])
                nc.sync.dma_start(xnat[:, :], in_ap)

                xT = tpp.tile([128, S], F32, tag="xT")
                for k in range(4):
                    pst = psY.tile([128, 128], F32, tag="yv")
                    nc.tensor.transpose(
                        pst[:, :], xnat[:, 128 * k: 128 * k + 128], ident)
                    nc.scalar.copy(xT[:, 128 * k: 128 * k + 128], pst[:, :])

                # ---- index math (fp32, [128, S]) ----
                ic = scp.tile([128, S], F32, tag="ic")
                icc = scp.tile([128, S], F32, tag="icc")
                ili = scp.tile([128, S], I16, tag="ili")
                ilf = scp.tile([128, S], F32, tag="ilf")
                fd = scp.tile([128, S], F32, tag="fx")
                il2 = scp.tile([128, S], F32, tag="il2")
                dd = scp.tile([128, S], F32, tag="fx")
                fu = scp.tile([128, S], F32, tag="fu")
                il3 = scp.tile([128, S], F32, tag="ilf")
                ff = scp2.tile([128, S], F32, tag="ff")

                nc.vector.tensor_scalar(ic[:, :], xT[:, :], 3.0, 682.5, A.add, A.mult)
                nc.vector.tensor_scalar(icc[:, :], ic[:, :], 0.0, 4095.0, A.max, A.min)
                nc.vector.tensor_copy(ili[:, :], icc[:, :])
                nc.vector.tensor_copy(ilf[:, :], ili[:, :])
                nc.vector.tensor_tensor(fd[:, :], ilf[:, :], icc[:, :], A.is_gt)
                nc.vector.tensor_tensor(il2[:, :], ilf[:, :], fd[:, :], A.subtract)
                nc.vector.tensor_tensor(dd[:, :], icc[:, :], il2[:, :], A.subtract)
                nc.vector.tensor_scalar(fu[:, :], dd[:, :], 1.0, None, A.is_ge)
                nc.vector.tensor_tensor(il3[:, :], il2[:, :], fu[:, :], A.add)
                nc.vector.tensor_tensor(ff[:, :], ic[:, :], il3[:, :], A.subtract)

                # ---- h = floor(il3/64), l = il3 - 64h ----
                hr = scp.tile([128, S], F32, tag="hr")
                hi = scp.tile([128, S], I16, tag="ili")
                hf = scp.tile([128, S], F32, tag="hf")
                hgt = scp.tile([128, S], F32, tag="fx")
                hff = scp.tile([128, S], F32, tag="hff")
                lf = scp.tile([128, S], F32, tag="lf")
                hb = scp2.tile([128, S], BF16, tag="hb")
                lb = scp2.tile([128, S], BF16, tag="lb")

                nc.vector.tensor_scalar(hr[:, :], il3[:, :], 0.015625, None, A.mult)
                nc.vector.tensor_copy(hi[:, :], hr[:, :])
                nc.vector.tensor_copy(hf[:, :], hi[:, :])
                nc.vector.tensor_tensor(hgt[:, :], hf[:, :], hr[:, :], A.is_gt)
                nc.vector.tensor_tensor(hff[:, :], hf[:, :], hgt[:, :], A.subtract)
                nc.vector.scalar_tensor_tensor(
                    lf[:, :], hff[:, :], -64.0, il3[:, :], A.mult, A.add)
                nc.vector.tensor_copy(hb[:, :], hff[:, :])
                nc.vector.tensor_copy(lb[:, :], lf[:, :])

                # ---- per-waveshaper one-hot gather, software-pipelined ----
                # A(w): broadcasts + one-hot prep (2 iterations ahead)
                # B(w): stage-1 matmuls + masked-mult
                # C(w): reductions (1 iteration behind)
                Vacc = accp.tile([128, S], F32, tag="vacc")
                Dacc = accp.tile([128, S], F32, tag="dacc")

                def emit_A(w):
                    Bh = psA.tile([128, S], F32, tag="bh")
                    nc.tensor.matmul(Bh[:, :], blk(0, w), hb[:, :])
                    SqH = sbp.tile([128, S], BF16, tag="sqh")
                    nc.scalar.activation(
                        SqH[:, :], Bh[:, :], AF.Square, bias=negiota, scale=1.0)
                    OH = sbp3.tile([128, S], BF16, tag="oh")
                    nc.vector.tensor_scalar(
                        OH[:, :], SqH[:, :], 0.5, None, A.is_lt)
                    Bl = psL.tile([128, S], F32, tag="bl")
                    nc.tensor.matmul(Bl[:, :], blk(0, w), lb[:, :])
                    SqL = sbp3.tile([128, S], BF16, tag="sql")
                    nc.scalar.activation(
                        SqL[:, :], Bl[:, :], AF.Square, bias=negiota, scale=1.0)
                    return OH, SqL

                def emit_B(w, OH, SqL):
                    Yv = psY.tile([128, S], F32, tag="yv")
                    nc.tensor.matmul(Yv[:, :], blk(1, w), OH[:, :])
                    Yd = psY2.tile([128, S], F32, tag="yd")
                    nc.tensor.matmul(Yd[:, :], blk(2, w), OH[:, :])
                    Zv = sbp.tile([128, S], BF16, tag="zv")
                    nc.vector.scalar_tensor_tensor(
                        Zv[:, :], SqL[:, :], 0.5, Yv[:, :], A.is_lt, A.mult)
                    Zd = sbp.tile([128, S], BF16, tag="zd")
                    nc.vector.scalar_tensor_tensor(
                        Zd[:, :], SqL[:, :], 0.5, Yd[:, :], A.is_lt, A.mult)
                    return Zv, Zd

                def emit_C(w, Zv, Zd):
                    nc.tensor.matmul(Vacc[:, :], blk(3, w), Zv[:, :],
                                     start=(w == 0), stop=(w == W - 1))
                    nc.tensor.matmul(Dacc[:, :], blk(3, w), Zd[:, :],
                                     start=(w == 0), stop=(w == W - 1))

                astash = {0: emit_A(0), 1: emit_A(1)}
                zstash = {}
                for w in range(W):
                    zstash[w] = emit_B(w, *astash.pop(w))
                    if w >= 1:
                        emit_C(w - 1, *zstash.pop(w - 1))
                    if w + 2 < W:
                        astash[w + 2] = emit_A(w + 2)
                emit_C(W - 1, *zstash.pop(W - 1))

                # ---- out = V + f*D, transpose back, store ----
                fdm = scp.tile([128, S], F32, tag="fdm")
                nc.vector.tensor_tensor(fdm[:, :], ff[:, :], Dacc[:, :], A.mult)
                outT = tpp.tile([128, S], F32, tag="outT")
                nc.vector.tensor_tensor(outT[:, :], fdm[:, :], Vacc[:, :], A.add)

                onat = iop.tile([128, 8 * W], F32, tag="onat")
                for k in range(4):
                    pst = psY.tile([128, 128], F32, tag="yv")
                    nc.tensor.transpose(
                        pst[:, :], outT[:, 128 * k: 128 * k + 128], ident)
                    nc.scalar.copy(onat[:, 128 * k: 128 * k + 128], pst[:, :])

                out_ap = bass.AP(y_d, base, [[8 * W, 128], [1, 8 * W]])
                nc.sync.dma_start(out_ap, onat[:, :])

            if group is None:
                for it in range(n_tiles):
                    emit_tile(it * (TROWS * W))
            else:
                assert n_tiles % group == 0
                step = group * TROWS * W
                with tc.For_i(0, (n_tiles // group) * step, step) as g:
                    for j in range(group):
                        emit_tile(g + j * (TROWS * W))

    if spill:
        _spill_waits(nc)
    return nc


def make_tables(lookup_table):
    """Build (tf_f32 [128,130], tb_bf16 [128, 256*128]) host-side tables."""
    import ml_dtypes
    lut = np.asarray(lookup_table, dtype=np.float32)          # [4096, 64]
    vu = np.concatenate([lut[1:], lut[-1:]], axis=0)
    delta = vu - lut                                          # [4096, 64]

    # T2v[w, h, l] = lut[64h + l, w]
    T2v = lut.reshape(64, 64, 64).transpose(2, 0, 1)
    T2d = delta.reshape(64, 64, 64).transpose(2, 0, 1)

    SEL = np.zeros((64, 128, 128), np.float32)
    S1v = np.zeros((64, 128, 128), np.float32)
    S1d = np.zeros((64, 128, 128), np.float32)
    RED = np.zeros((64, 128, 128), np.float32)
    for w in range(64):
        SEL[w, w, :64] = 1.0
        SEL[w, w + 64, 64:] = 1.0
        S1v[w, :64, :64] = T2v[w]
        S1v[w, 64:, 64:] = T2v[w]
        S1d[w, :64, :64] = T2d[w]
        S1d[w, 64:, 64:] = T2d[w]
        RED[w, :64, w] = 1.0
        RED[w, 64:, w + 64] = 1.0

    cat = np.concatenate([SEL, S1v, S1d, RED], axis=0)        # [256, 128, 128]
    big = np.ascontiguousarray(
        cat.transpose(1, 0, 2)).reshape(128, TBL_BLOCKS * 128)
    tb = big.astype(ml_dtypes.bfloat16)

    eye = np.eye(128, dtype=np.float32)
    p = np.arange(128, dtype=np.float32) % 64
    tf = np.concatenate(
        [eye, p[:, None], -p[:, None]], axis=1).astype(np.float32)
    return tf, tb


_CACHE = {}


def kernel(inputs, lookup_table):
    x = np.ascontiguousarray(np.asarray(inputs, dtype=np.float32))
    B, T, Wx = x.shape
    assert Wx == W
    per_core_b = B // N_CORES
    n_rows = per_core_b * T
    tf, tb = make_tables(lookup_table)

    if n_rows not in _CACHE:
        n_tiles = n_rows // TROWS
        grp = GROUP if (n_tiles % GROUP == 0 and n_tiles > GROUP) else None
        _CACHE[n_rows] = build_kernel(n_rows, group=grp)
    nc = _CACHE[n_rows]

    from concourse import bass_utils
    shards = x.reshape(N_CORES, n_rows, W)
    in_maps = [{"x": shards[c], "tf": tf, "tb": tb} for c in range(N_CORES)]
    res = bass_utils.run_bass_kernel_spmd(
        nc, in_maps, core_ids=list(range(N_CORES)))
    out = np.stack([res.results[c]["y"] for c in range(N_CORES)], axis=0)
    return out.reshape(B, T, W)
